# revision 1
# baseline (speedup 1.0000x reference)
"""Polynomial features (degree 2) + linear layer, distributed over 8 TRN2 cores.

reference: A = [x, {x_i*x_j for i<=j}] (8384 coeffs); out = A @ W.T + b.

Device algorithm (per core, batch shard 4096, feature-on-partition layout):
  - pairs are enumerated by circular distance class s in 0..64:
      class s, lane p  ->  unordered pair {p, (p+s) % 128}
    (each unordered pair appears exactly once; s=64 lanes >=64 are dups
    with zeroed weights)
  - host ships 16 rotated copies of x^T (rot d: row p = feature (p+d)%128)
    for d in D = {0..8, 16, 24, 32, 40, 48, 56, 64}; every class s is one
    bf16 DVE tensor_mul of two rotations with b - a = s (the hardware only
    allows 32-aligned partition bases, so all ops are full 128-partition,
    base 0 - the rotations do the shifting)
  - 66 matmuls (1 linear chunk + 65 class chunks, K=128 each) accumulate
    into PSUM [64 outs, 512 batch]; W is permuted host-side to match;
    bias is added in the PSUM->SBUF copy (DVE tensor_scalar_add)
  - TPB instructions have a single sync-wait slot, but Tile emits multiple
    waits on slot-recycling instructions; _split_multiwaits() post-processes
    the BIR, hoisting extra waits onto injected same-engine NOPs
"""

import numpy as np
import ml_dtypes

INPUT_DIM = 128
OUTPUT_DIM = 64
BATCH = 32768
N_CORES = 8
B_CORE = BATCH // N_CORES  # 4096
TILE_B = 512
N_TILES = B_CORE // TILE_B  # 8

ROT_SET = [0, 1, 2, 3, 4, 5, 6, 7, 8, 16, 24, 32, 40, 48, 56, 64]
N_ROT = len(ROT_SET)
ROT_IDX = {d: i for i, d in enumerate(ROT_SET)}

import os

GPS_OP_IDS = tuple(
    int(v) for v in os.environ.get("K_GPS_OPS", "").split(",") if v != ""
)


def _class_ops():
    """(a, b) rotation pair per distance class s=0..64 with b - a = s."""
    ops = []
    for s in range(65):
        if s <= 8:
            a, b = 0, s
        else:
            k = (s - 1) // 8  # 1..7
            anchor = 8 * k + 8
            a, b = anchor - s, anchor
        assert a in ROT_SET and b in ROT_SET and b - a == s, (s, a, b)
        ops.append((a, b))
    return ops


CLASS_OPS = _class_ops()


def _build_device_weights(W, b):
    """Permute W [64, 8384] into the device K-block layout.

    Returns w_packed [128, 66*64]: block j (j=0 linear, j=1+s class s)
    lives at free columns [j*64, (j+1)*64), partition p = K row p.
    Class s row p -> pair {p, (p+s)%128}; s=64 rows p>=64 are zeroed dups.
    """
    W = np.asarray(W, np.float32)
    n = INPUT_DIM
    pair_off = {}
    c = 0
    for i in range(n):
        for j in range(i, n):
            pair_off[(i, j)] = c
            c += 1
    assert c == 8256

    Wd = np.zeros((66, 128, OUTPUT_DIM), np.float32)
    Wd[0] = W[:, 0:128].T  # linear block
    seen = set()
    for s in range(65):
        a, _bb = CLASS_OPS[s]
        for p in range(128):
            u = (p + a) % 128
            v = (p + a + s) % 128
            i, j = (u, v) if u <= v else (v, u)
            if (i, j) in seen:
                continue  # duplicate lane (s=64 second half)
            seen.add((i, j))
            Wd[1 + s, p] = W[:, 128 + pair_off[(i, j)]]
    assert len(seen) == 8256, len(seen)
    w_packed = np.ascontiguousarray(
        Wd.transpose(1, 0, 2).reshape(128, 66 * OUTPUT_DIM)
    ).astype(ml_dtypes.bfloat16)
    return w_packed, np.asarray(b, np.float32)


def _split_multiwaits(nc, mybir):
    """TPB instructions have one sync-wait slot; hoist extras onto NOPs."""
    import bass_rust

    n_split = 0
    for fn in nc.m.functions:
        for bb in fn.blocks:
            out = []
            changed = False
            for inst in bb.instructions:
                si = getattr(inst, "sync_info", None)
                if si is not None and si.on_wait and len(si.on_wait) > 1:
                    for w in si.on_wait[:-1]:
                        n_split += 1
                        nop = bass_rust.InstNoOp(
                            name=f"I-mw{n_split}",
                            engine=inst.engine,
                            ins=[],
                            outs=[],
                            sync_info=mybir.SyncInfo(on_wait=[w], on_update=[]),
                            bass_nofuse=True,
                        )
                        out.append(nop)
                    inst.sync_info = mybir.SyncInfo(
                        on_wait=[si.on_wait[-1]], on_update=si.on_update
                    )
                    changed = True
                out.append(inst)
            if changed:
                bb.instructions = out
    return n_split


def build(x, W, b):
    """Build the Bass graph and per-core input maps. Returns (nc, in_maps)."""
    import concourse.bass as bass
    import concourse.mybir as mybir
    from concourse import tile

    bf16 = mybir.dt.bfloat16
    f32 = mybir.dt.float32

    # ---- host preprocessing ----
    xT = np.ascontiguousarray(np.asarray(x, np.float32).T).astype(
        ml_dtypes.bfloat16
    )  # [128, 32768]
    # xall[p, i, n] = feature (p + ROT_SET[i]) % 128 of sample n
    xall = np.stack([np.roll(xT, -d, axis=0) for d in ROT_SET], axis=1)
    w_packed, bias = _build_device_weights(W, b)

    # ---- device graph ----
    nc = bass.Bass()
    x_in = nc.declare_dram_parameter(
        "xall", [N_TILES, 128, N_ROT, TILE_B], bf16, isOutput=False
    )
    w_in = nc.declare_dram_parameter("Wd", [128, 66 * 64], bf16, isOutput=False)
    b_in = nc.declare_dram_parameter("bias", [OUTPUT_DIM, 1], f32, isOutput=False)
    out_ext = nc.declare_dram_parameter(
        "outT", [OUTPUT_DIM, B_CORE], f32, isOutput=True
    )

    # multi-class ops: one per anchor family, constant-stride rotation APs:
    # op 0 = classes 0..8 (rot0 x rot 0..8), ops 1..7 = classes 8k+1..8k+8
    MC_OPS = [list(range(0, 9))] + [
        list(range(8 * k + 1, 8 * k + 9)) for k in range(1, 8)
    ]
    GPS_OPS = set(GPS_OP_IDS)  # op indices computed on GpSimd

    def rot_group_ap(xrt, classes):
        """[128, len(classes), TILE_B] APs (in0, in1)."""
        m = len(classes)
        us = [ROT_IDX[CLASS_OPS[s][0]] for s in classes]
        vs = [ROT_IDX[CLASS_OPS[s][1]] for s in classes]

        def mk(idx):
            if all(i == idx[0] for i in idx):
                return xrt[:, idx[0] : idx[0] + 1, :].to_broadcast(
                    [128, m, TILE_B]
                )
            d = idx[1] - idx[0]
            assert all(idx[j + 1] - idx[j] == d for j in range(m - 1)), idx
            return xrt[:, idx[0] :: d, :][:, 0:m, :]

        return mk(us), mk(vs)

    with tile.TileContext(nc) as tc:
        with (
            tc.tile_pool(name="consts", bufs=1) as consts,
            tc.tile_pool(name="xc", bufs=3) as xcp,
            tc.tile_pool(name="prod", bufs=4) as prodp,
            tc.tile_pool(name="prodg", bufs=5) as prodgp,
            tc.tile_pool(name="outp", bufs=3) as outp,
            tc.tile_pool(name="psum", bufs=2, space="PSUM") as psump,
        ):
            w_sb = consts.tile([128, 66 * 64], bf16)
            nc.sync.dma_start(w_sb[:], w_in[:])
            b_sb = consts.tile([OUTPUT_DIM, 1], f32)
            nc.sync.dma_start(b_sb[:], b_in[:])

            xc_tiles = [None] * (N_TILES + 2)

            def load_xc(t):
                if t >= N_TILES:
                    return
                xt = xcp.tile([128, N_ROT, TILE_B], bf16, tag="xc", name="xc_t")
                nc.sync.dma_start(xt[:], x_in[t][:])
                xc_tiles[t] = xt

            load_xc(0)
            load_xc(1)
            for t in range(N_TILES):
                load_xc(t + 2)
                xrt = xc_tiles[t]

                # acc halves: even classes + linear -> partitions 0:64
                # (array cols 0-63), odd classes -> partitions 64:128
                acc = psump.tile([128, TILE_B], f32, name="acc")
                nc.tensor.matmul(
                    acc[0:64, :],
                    w_sb[:, 0:64],
                    xrt[:, 0, :],
                    start=True,
                    stop=False,
                    tile_position=(0, 0),
                )
                first_odd = True
                for k, classes in enumerate(MC_OPS):
                    m = len(classes)
                    pool_k = prodgp if k in GPS_OPS else prodp
                    tag = ("prodg" if k in GPS_OPS else "prod") + str(m)
                    p_t = pool_k.tile(
                        [128, m, TILE_B], bf16, tag=tag, name="p_t"
                    )
                    in0, in1 = rot_group_ap(xrt, classes)
                    eng = nc.gpsimd if k in GPS_OPS else nc.vector
                    eng.tensor_mul(p_t[:], in0, in1)
                    views = [
                        (s, p_t[:, j, :]) for j, s in enumerate(classes)
                    ]
                    for s, rhs in views:
                        half = s % 2
                        blk = 1 + s
                        is_last_even = s == 64
                        is_last_odd = s == 63
                        nc.tensor.matmul(
                            acc[64 * half : 64 * half + 64, :],
                            w_sb[:, blk * 64 : (blk + 1) * 64],
                            rhs,
                            start=(half == 1 and first_odd),
                            stop=(is_last_even or is_last_odd),
                            tile_position=(0, 64 * half),
                        )
                        if half == 1:
                            first_odd = False

                # ACT evacuates both PSUM halves; accumulating DMA adds the
                # odd half into DRAM (keeps DVE free for products)
                o_t = outp.tile([OUTPUT_DIM, TILE_B], f32, tag="o", name="o_t")
                o2_t = outp.tile([OUTPUT_DIM, TILE_B], f32, tag="o2", name="o2_t")
                nc.scalar.activation(
                    o_t[:],
                    acc[0:64, :],
                    mybir.ActivationFunctionType.Identity,
                    bias=b_sb[:, 0:1],
                )
                nc.scalar.copy(o2_t[:], acc[64:128, :])
                bs = slice(t * TILE_B, (t + 1) * TILE_B)
                nc.sync.dma_start(out_ext[:, bs], o_t[:])
                nc.gpsimd.dma_start(
                    out_ext[:, bs], o2_t[:], accum_op=mybir.AluOpType.add
                )

    _split_multiwaits(nc, mybir)

    # ---- per-core input maps ----
    in_maps = []
    for c in range(N_CORES):
        cs = xall[:, :, c * B_CORE : (c + 1) * B_CORE]  # [128, 16, 4096]
        xtiles = np.ascontiguousarray(
            cs.reshape(128, N_ROT, N_TILES, TILE_B).transpose(2, 0, 1, 3)
        )  # [N_TILES, 128, 16, TILE_B]
        in_maps.append(
            {
                "xall": xtiles,
                "Wd": w_packed,
                "bias": bias.reshape(OUTPUT_DIM, 1),
            }
        )
    return nc, in_maps


def kernel(x, W, b, indices_0, indices_1):
    from concourse.bass_utils import run_bass_kernel_spmd

    nc, in_maps = build(x, W, b)
    res = run_bass_kernel_spmd(nc, in_maps, list(range(N_CORES))).results
    out = np.concatenate([np.asarray(r["outT"], np.float32).T for r in res], axis=0)
    return out



# revision 2
# speedup vs baseline: 1.1487x; 1.1487x over previous
"""Polynomial features (degree 2) + linear layer, distributed over 8 TRN2 cores.

reference: A = [x, {x_i*x_j for i<=j}] (8384 coeffs); out = A @ W.T + b.

Pairs are enumerated by circular distance class s in 0..64:
  class s, lane p  ->  unordered pair {p, (p+s) % 128}
(each unordered pair appears exactly once; s=64 lanes >=64 are dups with
zeroed weights).

v2 three-engine split (per core, batch shard 4096, feature-on-partition):
  - classes 0..43 (DVE): host ships rotated copies of x^T in bf16; each
    DVE family op is one tensor_mul of two rotation groups (constant
    stride / broadcast APs, 2x perf mode)
  - classes 44..64 (PE+ACT): TensorE computes pair-SUMS via 0/1
    stationary matrices R_s (out[i] = x_i + x_{i+s}) into PSUM, ScalarE
    squares them (3 classes / op) into bf16 SBUF. Algebra:
    x_i*x_j = ((x_i+x_j)^2 - x_i^2 - x_j^2)/2 -> shifted-class weights
    are halved and the square corrections fold into the class-0 block.
  - 66 weight matmuls (1 linear + 65 class blocks, K=128 each) accumulate
    into PSUM [64 outs x 2 halves, 512 batch] via tile_position col
    packing; single ACT Identity (+bias rows 0:64) evacuates both halves;
    accumulating DMA folds the odd half into DRAM
  - TPB instructions have a single sync-wait slot; _split_multiwaits()
    hoists extra Tile-emitted waits onto injected same-engine NOPs
"""

import numpy as np
import ml_dtypes

INPUT_DIM = 128
OUTPUT_DIM = 64
BATCH = 32768
N_CORES = 8
B_CORE = BATCH // N_CORES  # 4096
TILE_B = 512
N_TILES = B_CORE // TILE_B  # 8

N_SHIFT = 21  # classes 65-N_SHIFT..64 via PE sums + ACT squares
SHIFT_START = 65 - N_SHIFT  # 44
SUM_GROUP = 3  # classes per PSUM sum-tile / ACT square op
N_GROUPS = N_SHIFT // SUM_GROUP  # 7

ROT_SET = [0, 1, 2, 3, 4, 5, 6, 7, 8, 16, 24, 32, 40, 48]
N_ROT_A = 9  # rots 0..8 -> chunk A (also feeds the sum matmuls via rot 0)
N_ROT_B = 5  # rots 16..48 -> chunk B (family anchors)
ROT_IDX = {d: i for i, d in enumerate(ROT_SET)}


def _class_ops():
    """(a, b) rotation pair per DVE distance class s with b - a = s."""
    ops = []
    for s in range(SHIFT_START):
        if s <= 8:
            a, b = 0, s
        else:
            k = (s - 1) // 8  # 1..5
            anchor = 8 * k + 8
            a, b = anchor - s, anchor
        assert a in ROT_SET and b in ROT_SET and b - a == s, (s, a, b)
        ops.append((a, b))
    return ops


CLASS_OPS = _class_ops()


def _dve_ops():
    """Group DVE classes into constant-stride family ops."""
    ops = [list(range(0, 9))]
    s = 9
    while s < SHIFT_START:
        e = min(s + 8, SHIFT_START)
        ops.append(list(range(s, e)))
        s = e
    return ops


DVE_OPS = _dve_ops()


def _build_device_weights(W, b):
    """Permute W [64, 8384] into the device K-block layout.

    Returns w_packed [128, 66*64]: block j (j=0 linear, j=1+s class s)
    at free columns [j*64, (j+1)*64), partition p = K row p. DVE class s
    row p -> pair {(p+a)%128, (p+a+s)%128}; shifted class s row p ->
    pair {p, (p+s)%128} with weight/2 and -w/2 corrections on class 0.
    Also returns r_packed [128, N_SHIFT*128] (0/1 pair-sum matrices) and
    the bias vector padded to 128 rows.
    """
    W = np.asarray(W, np.float32)
    n = INPUT_DIM
    pair_off = {}
    c = 0
    for i in range(n):
        for j in range(i, n):
            pair_off[(i, j)] = c
            c += 1
    assert c == 8256

    Wd = np.zeros((66, 128, OUTPUT_DIM), np.float32)
    Wd[0] = W[:, 0:128].T  # linear block
    seen = set()
    for s in range(65):
        a = CLASS_OPS[s][0] if s < SHIFT_START else 0
        scale = 0.5 if s >= SHIFT_START else 1.0
        for p in range(128):
            u = (p + a) % 128
            v = (p + a + s) % 128
            i, j = (u, v) if u <= v else (v, u)
            if (i, j) in seen:
                continue  # duplicate lane (s=64 second half)
            seen.add((i, j))
            w_pair = W[:, 128 + pair_off[(i, j)]]
            Wd[1 + s, p] = scale * w_pair
            if s >= SHIFT_START:
                Wd[1, i] -= 0.5 * w_pair
                Wd[1, j] -= 0.5 * w_pair
    assert len(seen) == 8256, len(seen)
    w_packed = np.ascontiguousarray(
        Wd.transpose(1, 0, 2).reshape(128, 66 * OUTPUT_DIM)
    ).astype(ml_dtypes.bfloat16)

    R = np.zeros((N_SHIFT, 128, 128), np.float32)
    for si in range(N_SHIFT):
        s = SHIFT_START + si
        for i in range(128):
            R[si, i, i] += 1.0
            R[si, (i + s) % 128, i] += 1.0
    r_packed = np.ascontiguousarray(
        R.transpose(1, 0, 2).reshape(128, N_SHIFT * 128)
    ).astype(ml_dtypes.bfloat16)

    bias = np.zeros((128, 1), np.float32)
    bias[0:OUTPUT_DIM, 0] = np.asarray(b, np.float32)
    return w_packed, r_packed, bias


def _split_multiwaits(nc, mybir):
    """TPB instructions have one sync-wait slot; hoist extras onto NOPs."""
    import bass_rust

    n_split = 0
    for fn in nc.m.functions:
        for bb in fn.blocks:
            out = []
            changed = False
            for inst in bb.instructions:
                si = getattr(inst, "sync_info", None)
                if si is not None and si.on_wait and len(si.on_wait) > 1:
                    for w in si.on_wait[:-1]:
                        n_split += 1
                        nop = bass_rust.InstNoOp(
                            name=f"I-mw{n_split}",
                            engine=inst.engine,
                            ins=[],
                            outs=[],
                            sync_info=mybir.SyncInfo(on_wait=[w], on_update=[]),
                            bass_nofuse=True,
                        )
                        out.append(nop)
                    inst.sync_info = mybir.SyncInfo(
                        on_wait=[si.on_wait[-1]], on_update=si.on_update
                    )
                    changed = True
                out.append(inst)
            if changed:
                bb.instructions = out
    return n_split


def build(x, W, b):
    """Build the Bass graph and per-core input maps. Returns (nc, in_maps)."""
    import concourse.bass as bass
    import concourse.mybir as mybir
    from concourse import tile

    bf16 = mybir.dt.bfloat16
    f32 = mybir.dt.float32

    # ---- host preprocessing ----
    xT = np.ascontiguousarray(np.asarray(x, np.float32).T).astype(
        ml_dtypes.bfloat16
    )  # [128, 32768]
    # xall[p, i, n] = feature (p + ROT_SET[i]) % 128 of sample n
    xall = np.stack([np.roll(xT, -d, axis=0) for d in ROT_SET], axis=1)
    w_packed, r_packed, bias = _build_device_weights(W, b)

    # ---- device graph ----
    nc = bass.Bass()
    xa_in = nc.declare_dram_parameter(
        "xa", [N_TILES, 128, N_ROT_A, TILE_B], bf16, isOutput=False
    )
    xb_in = nc.declare_dram_parameter(
        "xb", [N_TILES, 128, N_ROT_B, TILE_B], bf16, isOutput=False
    )
    w_in = nc.declare_dram_parameter("Wd", [128, 66 * 64], bf16, isOutput=False)
    r_in = nc.declare_dram_parameter(
        "Rd", [128, N_SHIFT * 128], bf16, isOutput=False
    )
    b_in = nc.declare_dram_parameter("bias", [128, 1], f32, isOutput=False)
    out_ext = nc.declare_dram_parameter(
        "outT", [OUTPUT_DIM, B_CORE], f32, isOutput=True
    )

    def rot_group_ap(xrt_a, xrt_b, classes):
        """[128, len(classes), TILE_B] APs (in0, in1) for one DVE op."""
        m = len(classes)
        us = [ROT_IDX[CLASS_OPS[s][0]] for s in classes]
        vs = [ROT_IDX[CLASS_OPS[s][1]] for s in classes]

        def mk(idx):
            # all a-rotations live in chunk A (idx 0..8); anchors in B
            if all(i == idx[0] for i in idx):
                src = xrt_a if idx[0] < N_ROT_A else xrt_b
                i0 = idx[0] if idx[0] < N_ROT_A else idx[0] - N_ROT_A
                return src[:, i0 : i0 + 1, :].to_broadcast([128, m, TILE_B])
            assert all(i < N_ROT_A for i in idx), idx
            if m == 1:
                return xrt_a[:, idx[0] : idx[0] + 1, :]
            d = idx[1] - idx[0]
            assert all(idx[j + 1] - idx[j] == d for j in range(m - 1)), idx
            return xrt_a[:, idx[0] :: d, :][:, 0:m, :]

        return mk(us), mk(vs)

    # weight-MM emission order per tile (for start/stop flags):
    # linear, shifted classes 44..64, then DVE classes op-by-op
    wmm_order = (
        ["lin"]
        + list(range(SHIFT_START, 65))
        + [s for cl in DVE_OPS for s in cl]
    )
    halves = {"lin": 0}
    for s in range(65):
        halves[s] = s % 2
    first_even = next(k for k in wmm_order if halves[k] == 0)
    first_odd = next(k for k in wmm_order if halves[k] == 1)
    last_even = next(k for k in reversed(wmm_order) if halves[k] == 0)
    last_odd = next(k for k in reversed(wmm_order) if halves[k] == 1)

    with tile.TileContext(nc) as tc:
        with (
            tc.tile_pool(name="consts", bufs=1) as consts,
            tc.tile_pool(name="xap", bufs=3) as xap,
            tc.tile_pool(name="xbp", bufs=3) as xbp,
            tc.tile_pool(name="prod", bufs=4) as prodp,
            tc.tile_pool(name="sq", bufs=4) as sqp,
            tc.tile_pool(name="outp", bufs=3) as outp,
            tc.tile_pool(name="acc", bufs=2, space="PSUM") as accp,
            tc.tile_pool(name="sums", bufs=2, space="PSUM") as sump,
        ):
            w_sb = consts.tile([128, 66 * 64], bf16)
            nc.sync.dma_start(w_sb[:], w_in[:])
            r_sb = consts.tile([128, N_SHIFT * 128], bf16)
            nc.sync.dma_start(r_sb[:], r_in[:])
            b_sb = consts.tile([128, 1], f32)
            nc.sync.dma_start(b_sb[:], b_in[:])

        # per-tile SBUF inputs
            xa_tiles = [None] * (N_TILES + 2)
            xb_tiles = [None] * (N_TILES + 2)

            def load_x(t):
                if t >= N_TILES:
                    return
                xt = xap.tile([128, N_ROT_A, TILE_B], bf16, tag="xa", name="xa_t")
                nc.sync.dma_start(xt[:], xa_in[t][:])
                xa_tiles[t] = xt
                xbt = xbp.tile([128, N_ROT_B, TILE_B], bf16, tag="xb", name="xb_t")
                nc.sync.dma_start(xbt[:], xb_in[t][:])
                xb_tiles[t] = xbt

            load_x(0)
            load_x(1)
            for t in range(N_TILES):
                load_x(t + 2)
                xrt_a = xa_tiles[t]
                xrt_b = xb_tiles[t]
                x0 = xrt_a[:, 0, :]

                acc = accp.tile([128, TILE_B], f32, name="acc")

                def wmm(key, rhs):
                    half = halves[key]
                    blk = 0 if key == "lin" else 1 + key
                    nc.tensor.matmul(
                        acc[64 * half : 64 * half + 64, :],
                        w_sb[:, blk * 64 : (blk + 1) * 64],
                        rhs,
                        start=(key == first_even or key == first_odd),
                        stop=(key == last_even or key == last_odd),
                        tile_position=(0, 64 * half),
                    )

                wmm("lin", x0)

                # shifted classes: PE sums -> ACT squares -> weight MMs.
                # pipeline: sums g0, g1; then (sq-wMMs g-1 | sums g+1)
                sum_tiles = [None] * N_GROUPS
                sq_tiles = [None] * N_GROUPS

                def emit_sums(g):
                    ps = sump.tile(
                        [128, SUM_GROUP, TILE_B], f32, tag="sums", name="sums"
                    )
                    for j in range(SUM_GROUP):
                        c = g * SUM_GROUP + j
                        nc.tensor.matmul(
                            ps[:, j, :],
                            r_sb[:, c * 128 : (c + 1) * 128],
                            x0,
                            start=True,
                            stop=True,
                        )
                    sum_tiles[g] = ps

                def emit_square(g):
                    sq = sqp.tile(
                        [128, SUM_GROUP, TILE_B], bf16, tag="sq", name="sq"
                    )
                    nc.scalar.activation(
                        sq[:],
                        sum_tiles[g][:],
                        mybir.ActivationFunctionType.Square,
                    )
                    sq_tiles[g] = sq

                def emit_sq_wmms(g):
                    for j in range(SUM_GROUP):
                        s = SHIFT_START + g * SUM_GROUP + j
                        wmm(s, sq_tiles[g][:, j, :])

                def emit_dve(k):
                    classes = DVE_OPS[k]
                    m = len(classes)
                    p_t = prodp.tile(
                        [128, m, TILE_B], bf16, tag=f"prod{m}", name="p_t"
                    )
                    in0, in1 = rot_group_ap(xrt_a, xrt_b, classes)
                    nc.vector.tensor_mul(p_t[:], in0, in1)
                    for j, s in enumerate(classes):
                        wmm(s, p_t[:, j, :])

                # interleave: keep PE fed from (sums, sq-wMMs) while DVE
                # families arrive; squares double-buffered through PSUM
                emit_sums(0)
                emit_square(0)
                emit_sums(1)
                emit_sq_wmms(0)
                emit_square(1)
                emit_sums(2)
                emit_sq_wmms(1)
                emit_square(2)
                emit_sums(3)
                emit_dve(0)
                emit_sq_wmms(2)
                emit_square(3)
                emit_sums(4)
                emit_sq_wmms(3)
                emit_square(4)
                emit_sums(5)
                emit_dve(1)
                emit_sq_wmms(4)
                emit_square(5)
                emit_sums(6)
                emit_sq_wmms(5)
                emit_square(6)
                emit_dve(2)
                emit_sq_wmms(6)
                emit_dve(3)
                emit_dve(4)
                emit_dve(5)

                # single ACT evacuates both halves (+bias on rows 0:64);
                # accumulating DMA folds the odd half into DRAM
                o_t = outp.tile([128, TILE_B], f32, tag="o", name="o_t")
                nc.scalar.activation(
                    o_t[:],
                    acc[:],
                    mybir.ActivationFunctionType.Identity,
                    bias=b_sb[:, 0:1],
                )
                bs = slice(t * TILE_B, (t + 1) * TILE_B)
                nc.sync.dma_start(out_ext[:, bs], o_t[0:64, :])
                nc.gpsimd.dma_start(
                    out_ext[:, bs], o_t[64:128, :], accum_op=mybir.AluOpType.add
                )

    _split_multiwaits(nc, mybir)

    # ---- per-core input maps ----
    in_maps = []
    for c in range(N_CORES):
        cs = xall[:, :, c * B_CORE : (c + 1) * B_CORE]  # [128, 14, 4096]
        xtiles = np.ascontiguousarray(
            cs.reshape(128, len(ROT_SET), N_TILES, TILE_B).transpose(2, 0, 1, 3)
        )  # [N_TILES, 128, 14, TILE_B]
        in_maps.append(
            {
                "xa": np.ascontiguousarray(xtiles[:, :, 0:N_ROT_A, :]),
                "xb": np.ascontiguousarray(xtiles[:, :, N_ROT_A:, :]),
                "Wd": w_packed,
                "Rd": r_packed,
                "bias": bias,
            }
        )
    return nc, in_maps


def kernel(x, W, b, indices_0, indices_1):
    from concourse.bass_utils import run_bass_kernel_spmd

    nc, in_maps = build(x, W, b)
    res = run_bass_kernel_spmd(nc, in_maps, list(range(N_CORES))).results
    out = np.concatenate([np.asarray(r["outT"], np.float32).T for r in res], axis=0)
    return out
